# revision 15
# baseline (speedup 1.0000x reference)
"""Bayesian GPLVM collapsed-ELBO kernel for Trainium2 (8 NeuronCores).

Sharding: data-parallel over n (2048 rows -> 256 per core). All O(n*q)
row prep (softplus, d1/d2, w1/w2, log-sums, KL pieces) is done on host
in float64 and shipped as small per-core operand blocks; the device does
only the O(n*m) / O(n*m^2) work:

  - psi1 exponent = p1^T @ zs1 (two 33x128x64 matmuls into one PSUM
    tile), one Exp, then A += psi1^T y accumulated in PSUM; these slot
    into the psi2 matmul train once their inputs land.
  - psi2: for each of 17 ij-chunks (128 upper-triangle pairs each),
    exponent = zl_chunk^T @ p2 (34x128x256, bf16), Exp on ACT, free-axis
    n-sum on DVE; the last chunk's n-sum rides the Exp via accum_out.
    Host sums the 8 per-core partials and does the small m x m algebra.

DMA plan: the psi2 stream (zl, 4 slices) goes down the sync HWDGE
queue while the psi1-side blocks (p2/p1z/yb) go down the scalar HWDGE
queue, so descriptor generation is parallel and the first psi2 matmul
can start as early as possible. All outputs drain via sync. Only the
Exp table is ever needed, so there is a single ACT_TABLE_LOAD.
"""

import numpy as np

N, D, Q, M = 2048, 256, 16, 64
NCORES = 8
NLOC = N // NCORES          # 256
NPAIRS = 2080               # upper-triangle pairs of 64x64
NCHUNK = 17                 # ceil(2080 / 128)
# (start_chunk, n_chunks) per PSUM group; last chunk of the last group
# is summed via accum_out instead of a DVE reduce
GROUPS = [(0, 4), (4, 4), (8, 4), (12, 3), (15, 2)]
ZL_SLICES = [(0, 384), (384, 896), (896, 1536), (1536, 2176)]
HALF = 128

_compiled = None


def _build_bass():
    import concourse.bacc as bacc
    import concourse.bass as bass  # noqa: F401
    import concourse.mybir as mybir
    from concourse.tile import TileContext

    f32 = mybir.dt.float32
    f32r = mybir.dt.float32r
    bf16 = mybir.dt.bfloat16
    AF = mybir.ActivationFunctionType
    OP = mybir.AluOpType

    nc = bacc.Bacc("TRN2", target_bir_lowering=False, num_swdge_queues=2)

    p1z_d = nc.declare_dram_parameter("p1z", [33, 320], f32r, isOutput=False)
    p2_d = nc.declare_dram_parameter("p2", [34, NLOC], bf16, isOutput=False)
    y_d = nc.declare_dram_parameter("yb", [128, 512], f32r, isOutput=False)
    zl_d = nc.declare_dram_parameter("zl", [34, NCHUNK * 128], bf16,
                                     isOutput=False)
    psi2_o = nc.declare_dram_parameter("out_psi2", [128, NCHUNK], f32,
                                       isOutput=True)
    a_o = nc.declare_dram_parameter("out_A", [M, D], f32, isOutput=True)

    with TileContext(nc) as tc:
        with (
            tc.tile_pool(name="const", bufs=1) as cpool,
            tc.tile_pool(name="scr", bufs=2) as spool,
            tc.tile_pool(name="psum", bufs=3, space="PSUM") as ppool,
            tc.tile_pool(name="psume", bufs=1, space="PSUM") as ppool_e,
            tc.tile_pool(name="psuma", bufs=1, space="PSUM") as ppool_a,
        ):
            p1z = cpool.tile([33, 320], f32r)
            p2 = cpool.tile([34, NLOC], bf16)
            yb = cpool.tile([128, 512], f32r)
            zl = cpool.tile([34, NCHUNK * 128], bf16)
            stats = cpool.tile([128, NCHUNK], f32)
            psi1c = cpool.tile([128, 128], f32r)
            a_sb = cpool.tile([M, D], f32)

            # psi2 stream (p2 + zl) on the sync HWDGE queue, psi1-side
            # blocks on the scalar HWDGE queue (parallel desc-gen)
            nc.sync.dma_start(out=p2[:, :], in_=p2_d[:, :])
            for c0, c1 in ZL_SLICES:
                nc.sync.dma_start(out=zl[:, c0:c1], in_=zl_d[:, c0:c1])
            nc.scalar.dma_start(out=p1z[:, :], in_=p1z_d[:, :])
            nc.scalar.dma_start(out=yb[:, :], in_=y_d[:, :])

            e1 = ppool_e.tile([128, 128], f32)
            apsum = ppool_a.tile([M, D], f32)

            def e1_matmuls():
                for c in range(2):
                    nc.tensor.matmul(e1[:, c * 64:(c + 1) * 64],
                                     lhsT=p1z[:, c * 128:(c + 1) * 128],
                                     rhs=p1z[:, 256:320],
                                     start=True, stop=True)

            def a_matmuls():
                for c in range(2):
                    nc.tensor.matmul(apsum[:, :],
                                     lhsT=psi1c[:, c * 64:(c + 1) * 64],
                                     rhs=yb[:, c * 256:(c + 1) * 256],
                                     start=(c == 0), stop=(c == 1))

            # psi2 exponent matmul train; e1 slots in after chunk 4,
            # A after the full train (psi1c/yb surely landed by then)
            ptiles = []
            for ch0, nch in GROUPS:
                p2p = ppool.tile([128, 4 * NLOC], f32, tag="p2p")
                ptiles.append((p2p, ch0, nch))
                for j in range(nch):
                    ch = ch0 + j
                    nc.tensor.matmul(
                        p2p[:, j * NLOC:(j + 1) * NLOC],
                        lhsT=zl[:, ch * 128:(ch + 1) * 128],
                        rhs=p2[:, :],
                        start=True, stop=True)
                    if ch == 4:
                        e1_matmuls()

            exps = []
            for gi, (p2p, ch0, nch) in enumerate(ptiles):
                scr = spool.tile([128, 4 * NLOC], bf16, tag="scr")
                half = spool.tile([128, 4 * HALF], bf16, tag="half")
                exps.append((p2p, scr, half, ch0, nch))

            def do_group(gi, last_accum=False):
                p2p, scr, half, ch0, nch = exps[gi]
                nred = nch - 1 if last_accum else nch
                if nred:
                    nc.scalar.activation(scr[:, :nred * NLOC],
                                         p2p[:, :nred * NLOC], AF.Exp)
                if last_accum:
                    w0, w1 = nred * NLOC, nch * NLOC
                    nc.scalar.activation(scr[:, w0:w1], p2p[:, w0:w1],
                                         AF.Exp,
                                         accum_out=stats[:, ch0 + nred:
                                                         ch0 + nred + 1])
                return nred

            def do_reduce(gi, nred):
                # bf16 pair-sum (DVE 2x fast path) then f32 reduce
                p2p, scr, half, ch0, nch = exps[gi]
                v = scr[:, :nred * NLOC].rearrange("p (a b) -> p a b",
                                                   b=NLOC)
                nc.vector.tensor_tensor(
                    half[:, :nred * HALF].rearrange("p (a b) -> p a b",
                                                    b=HALF),
                    v[:, :, 0:HALF], v[:, :, HALF:NLOC], op=OP.add)
                nc.vector.tensor_reduce(
                    stats[:, ch0:ch0 + nred],
                    half[:, :nred * HALF].rearrange("p (a b) -> p a b",
                                                    b=HALF),
                    axis=mybir.AxisListType.X, op=OP.add)

            # ACT chain: e0, e1g, psi1-exp, e2, e3, e4+accum; DVE runs
            # the reduces with the A copy slotted after r1
            do_group(0)
            do_reduce(0, 4)
            nc.sync.dma_start(out=psi2_o[:, 0:4], in_=stats[:, 0:4])

            do_group(1)
            nc.scalar.activation(psi1c[:, :], e1[:, :], AF.Exp)
            a_matmuls()
            do_reduce(1, 4)
            nc.vector.tensor_copy(a_sb[:, :], apsum[:, :])
            nc.sync.dma_start(out=psi2_o[:, 4:8], in_=stats[:, 4:8])
            nc.sync.dma_start(out=a_o[:, :], in_=a_sb[:, :])

            do_group(2)
            do_reduce(2, 4)
            nc.sync.dma_start(out=psi2_o[:, 8:12], in_=stats[:, 8:12])

            do_group(3)
            do_reduce(3, 3)
            nc.sync.dma_start(out=psi2_o[:, 12:15], in_=stats[:, 12:15])

            do_group(4, last_accum=True)
            do_reduce(4, 1)
            nc.sync.dma_start(out=psi2_o[:, 15:17], in_=stats[:, 15:17])

    nc.compile()
    return nc


def _get_compiled():
    global _compiled
    if _compiled is None:
        _compiled = _build_bass()
    return _compiled


def kernel(y, q_mu, q_log_sigma, z, noise_raw, alpha, variance, _trace=False):
    import ml_dtypes
    from concourse.bass_utils import run_bass_kernel_spmd

    nc = _get_compiled()

    f8 = np.float64
    qm = q_mu.astype(f8)
    qls = q_log_sigma.astype(f8)
    z64 = z.astype(f8)
    al = alpha.astype(f8)
    var = f8(variance[0])
    logvar = np.log(var)

    # ---- host row prep (O(n*q)) ----
    qsig = np.logaddexp(qls, 0.0)                           # softplus
    d1 = qsig * al + 1.0
    d2 = 2.0 * al * qsig + 1.0
    w1 = al / d1
    w2 = al / d2
    lse1 = np.sum(np.log(d1), axis=1)                       # (n,)
    lse2 = np.sum(np.log(d2), axis=1)
    rt1 = np.sum(qm * qm * w1, axis=1)
    rt2 = np.sum(qm * qm * w2, axis=1)
    h1 = 2.0 * logvar - 0.5 * (rt1 + lse1)
    g = 4.0 * logvar - rt2 - 0.5 * lse2

    kl_sum = np.sum(-np.log(qsig) + 0.5 * (qsig * qsig + qm * qm - 1.0))
    tr_yy = np.sum(y.astype(f8) ** 2)

    # ---- z-side blocks (replicated) ----
    iu, ju = np.triu_indices(M)                             # (2080,)
    Su = z64[iu] + z64[ju]                                  # (2080, q)
    sqz = (z64[:, None, :] - z64[None, :, :]) ** 2          # (m, m, q)
    s1 = 0.25 * (sqz @ al)                                  # (m, m)
    zl = np.zeros((34, NCHUNK * 128), np.float32)
    zl[0:16, :NPAIRS] = Su.T
    zl[16:32, :NPAIRS] = (-0.25 * Su * Su).T
    zl[32, :NPAIRS] = 1.0
    zl[33, :NPAIRS] = -s1[iu, ju]
    zl = zl.astype(ml_dtypes.bfloat16)

    zt = z64.T                                              # (q, m)

    in_maps = []
    for i in range(NCORES):
        sl = slice(i * NLOC, (i + 1) * NLOC)
        p1z = np.zeros((33, 320), np.float32)
        p1z[0:16, 0:NLOC] = (qm[sl] * w1[sl]).T
        p1z[16:32, 0:NLOC] = w1[sl].T
        p1z[32, 0:NLOC] = h1[sl]
        p1z[0:16, 256:320] = zt
        p1z[16:32, 256:320] = -0.5 * zt * zt
        p1z[32, 256:320] = 1.0

        p2 = np.empty((34, NLOC), np.float32)
        p2[0:16] = (qm[sl] * w2[sl]).T
        p2[16:32] = w2[sl].T
        p2[32] = g[sl]
        p2[33] = 1.0

        yb = np.ascontiguousarray(
            y[sl].astype(np.float32).reshape(2, 128, D)
            .transpose(1, 0, 2).reshape(128, 512))

        in_maps.append({
            "p1z": p1z,
            "p2": p2.astype(ml_dtypes.bfloat16),
            "yb": yb,
            "zl": zl,
        })

    br = run_bass_kernel_spmd(nc, in_maps, list(range(NCORES)), trace=_trace)
    res = br.results

    psi2_part = np.zeros((128, NCHUNK), f8)
    A = np.zeros((M, D), f8)
    for r in res:
        psi2_part += r["out_psi2"].astype(f8)
        A += r["out_A"].astype(f8)

    flat = psi2_part.T.reshape(NCHUNK * 128)
    psi2 = np.empty((M, M), f8)
    psi2[iu, ju] = flat[:NPAIRS]
    psi2[ju, iu] = flat[:NPAIRS]

    kl_term = kl_sum / (N * D)

    # small m x m algebra on host
    k_mm = var * np.exp(-0.5 * (sqz @ al))                  # (m, m)
    noise_var = np.logaddexp(f8(noise_raw[0]), 0.0)
    beta = 1.0 / noise_var
    psi0 = N * var

    cov1 = beta * psi2 + k_mm
    B = np.linalg.solve(cov1, A)
    tr_yWy = beta * tr_yy - np.sum(A * B)

    F = 0.5 * N * np.log(beta)
    F += 0.5 * np.linalg.slogdet(k_mm)[1]
    F -= 0.5 * N * np.log(np.pi)
    F -= 0.5 * np.linalg.slogdet(cov1)[1]
    F -= 0.5 * beta * psi0
    F += 0.5 * np.trace(np.linalg.solve(k_mm, psi2))
    F = (F * D - 0.5 * tr_yWy) / (N * D)

    out = F - kl_term
    result = np.asarray(out, dtype=np.float32)
    if _trace:
        return result, br
    return result
